# revision 20
# baseline (speedup 1.0000x reference)
"""Trainium2 Bass kernel for Swin-style windowed cosine attention.

Problem: nn_Attention_8100308321041
  q,k,v: [512, 8, 256, 16] f32; table: [961, 8]; index: [65536] i64;
  mask: [64, 256, 256] f32; out: [512, 256, 128] f32.

Strategy (8 NeuronCores, pure data-parallel):
  - Shard window-instances b by (b % 64) % 8 == core -> 64 instances/core,
    ordered (wl, img) so each per-window bias+mask table chunk is fetched
    once and reused across 8 images.
  - Host prep: l2-normalize q/k -> bf16 4-head row-group layout (partition
    32*g + d), concatenated with v_aug (ones column -> AV emits numerators
    AND softmax denominators; final divide on HOST, raw av dump shipped
    back bf16).  Per-window tables: CbE = exp(bias+mask) bf16 (pairs
    0,1,3), Cp = round(A*(bias+mask)+B) int16 (pairs 2,3).
  - Device per instance, exp paths balanced across engines (PE kept at its
    4640-cycle floor -- no identity preloads):
      pair0: MULT_ACT  ScalarE exp(S) + GpSimd bf16 multiply by exp(C)
      pair1: MULT_ACT  ScalarE exp(S) + VectorE bf16 2x multiply
      pair2: STT       VectorE Schraudolph int16 bitcast (C-add fused)
      pair3: STT even insts / MULT_ACT (VectorE mult) odd
    out dump copy on ScalarE; qkv + out + C DMA issue on SP queue.
"""

import os
import sys

sys.path.insert(0, "/opt/trn_rl_repo")

import numpy as np
import ml_dtypes

import concourse.bass as bass
import concourse.bacc as bacc
import concourse.mybir as mybir
from concourse import tile
from concourse.bass_utils import run_bass_kernel_spmd

BF16 = ml_dtypes.bfloat16

B_, H, N, D = 512, 8, 256, 16
NW = 64          # windows per image
M_CORES = 8
IMG = B_ // NW   # 8 images
WL = NW // M_CORES  # 8 distinct windows per core
NI = IMG * WL    # 64 instances per core
HD = H * D       # 128
EPS = 1e-12
CBE_WL = 3 * 2 * 2 * N   # exp(C) cols per wl: pairs 0,1,3        (3072)
CP_WL = 2 * 2 * 2 * N    # Schraudolph C' cols per wl: pairs 2,3  (2048)
QKC = 2 * 2 * N          # qk cols per inst (1024)
VC = 2 * H * 17          # v_aug cols per inst (272)
A16 = 128.0 / float(np.log(2.0))     # Schraudolph scale for bf16-via-int16
B16 = 127.0 * 128.0 - 5.09           # Schraudolph bias (round-to-nearest c)

# per-pair exp paths
MULT_ACT, STT = 1, 2

_NC_CACHE = {}


def _paths(inst):
    return [
        MULT_ACT,
        MULT_ACT,
        STT,
        STT if inst % 2 == 0 else MULT_ACT,
    ]


def _cbe_off(wl, pr):
    slot = {0: 0, 1: 1, 3: 2}[pr]
    return wl * CBE_WL + slot * 1024


def _cp_off(wl, pr):
    assert pr in (2, 3)
    return wl * CP_WL + (pr - 2) * 1024


def build_bass(trace_sim=False):
    nc = bacc.Bacc("TRN2", target_bir_lowering=False, debug=False, num_devices=M_CORES)
    qkv = nc.declare_dram_parameter("qkv", [NI, 128, QKC + VC], mybir.dt.bfloat16, isOutput=False)
    CbE = nc.declare_dram_parameter("CbE", [128, WL * CBE_WL], mybir.dt.bfloat16, isOutput=False)
    Cp = nc.declare_dram_parameter("Cp", [128, WL * CP_WL], mybir.dt.int16, isOutput=False)
    out = nc.declare_dram_parameter("out", [NI, 128, VC], mybir.dt.bfloat16, isOutput=True)

    FP32 = mybir.dt.float32
    BF = mybir.dt.bfloat16
    I16 = mybir.dt.int16
    Exp = mybir.ActivationFunctionType.Exp
    Copy = mybir.ActivationFunctionType.Copy

    with tile.TileContext(nc, trace_sim=trace_sim) as tc:
        with (
            tc.tile_pool(name="const", bufs=1) as constp,
            tc.tile_pool(name="qkv", bufs=5) as qkvp,
            tc.tile_pool(name="pp", bufs=10) as ppool,
            tc.tile_pool(name="p0", bufs=6) as p0pool,
            tc.tile_pool(name="op", bufs=3) as opool,
            tc.tile_pool(name="ps", bufs=4, space=bass.MemorySpace.PSUM) as psp,
        ):
            cetile = constp.tile([128, WL * CBE_WL], BF)
            cptile = constp.tile([128, WL * CP_WL], I16)

            def fetch_c(wl):
                nc.sync.dma_start(cetile[:, wl * CBE_WL:(wl + 1) * CBE_WL], CbE[:, wl * CBE_WL:(wl + 1) * CBE_WL])
                nc.sync.dma_start(cptile[:, wl * CP_WL:(wl + 1) * CP_WL], Cp[:, wl * CP_WL:(wl + 1) * CP_WL])

            fetch_c(0)
            fetch_c(1)

            pending = []  # deferred AV + out work from the previous instance

            def emit_av(p_state):
                (p_inst, p_vt, p_pb, avps) = p_state
                for pr in range(4):
                    pbf = p_pb[pr]
                    for hh in range(2):
                        h = 2 * pr + hh
                        hoff = hh * 512
                        for nck in range(2):
                            for mc in range(2):
                                nc.tensor.matmul(
                                    avps[:, nck * (H * 17) + h * 17: nck * (H * 17) + h * 17 + 17],
                                    pbf[:, hoff + mc * 256 + nck * 128: hoff + mc * 256 + nck * 128 + 128],
                                    p_vt[:, mc * (H * 17) + h * 17: mc * (H * 17) + h * 17 + 17],
                                    start=(mc == 0), stop=(mc == 1),
                                )

            def emit_out(p_state):
                # bf16 copy of numerators+denominators on ScalarE; host divides.
                (p_inst, p_vt, p_pb, avps) = p_state
                otile = opool.tile([128, VC], BF, name="otile")
                nc.scalar.activation(otile[:], avps, Copy)
                nc.sync.dma_start(out[p_inst], otile[:])

            def fetch_inst(i):
                t = qkvp.tile([128, QKC + VC], BF, name="qkvtile")
                nc.sync.dma_start(t[:], qkv[i])
                return t

            inst_tiles = {0: fetch_inst(0), 1: fetch_inst(1), 2: fetch_inst(2)}

            for inst in range(NI):
                wl = inst // IMG
                if inst % IMG == 0 and wl + 2 < WL:
                    fetch_c(wl + 2)
                qkvtile = inst_tiles.pop(inst)
                if inst + 3 < NI:
                    inst_tiles[inst + 3] = fetch_inst(inst + 3)
                qk5 = qkvtile[:, 0:QKC].rearrange("p (s q n) -> p s q n", s=2, q=2)
                vtile = qkvtile[:, QKC:QKC + VC]

                paths = _paths(inst)

                pstiles = []
                for pr in range(4):
                    ps = psp.tile([128, 1024], FP32, name="ps", tag="ps")
                    pstiles.append(ps)
                avps_full = psp.tile([128, 1024], FP32, name="avps", tag="ps")

                def qk_burst(half):
                    for mc in range(2):
                        for g in range(4):
                            h = 4 * half + g
                            pr = h // 2
                            hoff = (h % 2) * 512
                            qkh = qk5[32 * g: 32 * g + D, half]
                            nc.tensor.matmul(
                                pstiles[pr][:, hoff + mc * 256: hoff + mc * 256 + 256],
                                qkh[:, 1, mc * 128:(mc + 1) * 128],
                                qkh[:, 0, :],
                                start=(mc == 0),
                                stop=(mc == 1),
                                skip_group_check=True,
                                tile_position=(32 * g, 0),
                            )

                def evac(pr):
                    if paths[pr] == STT:
                        ptile = ppool.tile([128, 1024], I16, name="pt", tag="pt")
                        nc.vector.scalar_tensor_tensor(
                            ptile[:], pstiles[pr][:], A16,
                            cptile[:, _cp_off(wl, pr): _cp_off(wl, pr) + 1024],
                            mybir.AluOpType.mult, mybir.AluOpType.add,
                        )
                        return ptile[:].bitcast(BF)
                    # MULT_ACT: ScalarE exp, then bf16 multiply by exp(C).
                    # pair0's multiply runs on GpSimd (SBUF-only op), the
                    # rest on VectorE.
                    p0tile = p0pool.tile([128, 1024], BF, name="p0t", tag="p0t")
                    nc.scalar.activation(p0tile[:], pstiles[pr][:], Exp)
                    ptile = ppool.tile([128, 1024], BF, name="pt", tag="pt")
                    eng = nc.gpsimd if pr == 0 else nc.vector
                    eng.tensor_mul(
                        ptile[:], p0tile[:],
                        cetile[:, _cbe_off(wl, pr): _cbe_off(wl, pr) + 1024],
                    )
                    return ptile[:]

                # QK half0 (pairs 0,1), evac them, then previous instance's
                # AV + out (before QK half1 so the pool rotation can never
                # deadlock), then QK half1 (pairs 2,3) + evac
                qk_burst(0)
                ptiles = [None] * 4
                ptiles[0] = evac(0)
                ptiles[1] = evac(1)
                if pending:
                    emit_av(pending[0])
                    emit_out(pending[0])
                    pending.clear()
                qk_burst(1)
                ptiles[2] = evac(2)
                ptiles[3] = evac(3)

                pending.append((inst, vtile, ptiles, avps_full[:, 0:VC]))

            if pending:
                emit_av(pending[0])
                emit_out(pending[0])
                pending.clear()
    nc.compile()
    return nc


def _host_prep(q, k, v, table, index, mask):
    """Returns per-core input maps + the inverse b-index map."""
    qn = q / np.maximum(np.sqrt((q * q).sum(-1, keepdims=True)), EPS)
    kn = k / np.maximum(np.sqrt((k * k).sum(-1, keepdims=True)), EPS)
    # 4-head row-group layout: [b, g, d(padded to 32), half, qk, n], h = 4*half+g
    qk8 = np.zeros((B_, 4, 32, 2, 2, N), np.float32)
    qk8[:, :, :D, :, 0] = qn.transpose(0, 1, 3, 2).reshape(B_, 2, 4, D, N).transpose(0, 2, 3, 1, 4)
    qk8[:, :, :D, :, 1] = kn.transpose(0, 1, 3, 2).reshape(B_, 2, 4, D, N).transpose(0, 2, 3, 1, 4)
    qk8 = qk8.reshape(B_, 128, QKC)
    # v_aug [b, n, h, 17] -> [b, mc, 128, h, 17] -> [b, 128, mc*h*17]
    vA = np.empty((B_, N, H, 17), np.float32)
    vA[..., :16] = v.transpose(0, 2, 1, 3)
    vA[..., 16] = 1.0
    vA = vA.reshape(B_, 2, 128, H * 17).transpose(0, 2, 1, 3).reshape(B_, 128, VC)
    qkv = np.concatenate([qk8, vA], axis=2).astype(BF16)  # [B_, 128, 1296]
    # bias'[h, m, n] = table[index[n*256+m], h]
    bias = table[index.astype(np.int64)].reshape(N, N, H).transpose(2, 1, 0)  # [h, m, n]
    maskT = mask.transpose(0, 2, 1)  # [w, m, n]

    in_maps = []
    b_order = []
    for c in range(M_CORES):
        bs = np.array([img * NW + (c + M_CORES * wl) for wl in range(WL) for img in range(IMG)])
        b_order.append(bs)
        C = (bias[None, :, :, :] + maskT[c::M_CORES][:, None, :, :]).astype(np.float32)
        C = C.reshape(WL, H, 2, 128, N)  # [wl, h, mc, 128, n]
        # exp(C) path: pairs {0,1,3} = heads 0,1,2,3,6,7, bf16
        CbE_ = np.exp(C[:, [0, 1, 2, 3, 6, 7]]).transpose(3, 0, 1, 2, 4).reshape(128, WL * CBE_WL).astype(BF16)
        # Schraudolph path: pairs {2,3} = heads 4..7, int16 pre-scaled A*C + B
        Cp_ = np.rint(A16 * C[:, 4:] + B16).transpose(3, 0, 1, 2, 4).reshape(128, WL * CP_WL).astype(np.int16)
        in_maps.append({
            "qkv": np.ascontiguousarray(qkv[bs]),
            "CbE": CbE_,
            "Cp": Cp_,
        })
    return in_maps, b_order


def kernel(q, k, v, table, index, mask):
    q = np.asarray(q, np.float32)
    k = np.asarray(k, np.float32)
    v = np.asarray(v, np.float32)
    table = np.asarray(table, np.float32)
    index = np.asarray(index)
    mask = np.asarray(mask, np.float32)

    in_maps, b_order = _host_prep(q, k, v, table, index, mask)

    if "nc" not in _NC_CACHE:
        _NC_CACHE["nc"] = build_bass()
    nc = _NC_CACHE["nc"]

    res = run_bass_kernel_spmd(nc, in_maps, core_ids=list(range(M_CORES)))
    out = np.empty((B_, N, HD), np.float32)
    for c in range(M_CORES):
        # av dump [NI, 128, (nck h x)] bf16: x = 16 numerators + denominator
        arr = res.results[c]["out"].astype(np.float32).reshape(NI, 128, 2, H, 17)
        o = arr[..., :16] / arr[..., 16:17]           # [NI, p, nck, H, D]
        out[b_order[c]] = o.transpose(0, 2, 1, 3, 4).reshape(NI, N, HD)
    return out


if __name__ == "__main__":
    rng = np.random.default_rng(0)
    q = rng.standard_normal((B_, H, N, D), dtype=np.float32)
    k = rng.standard_normal((B_, H, N, D), dtype=np.float32)
    v = rng.standard_normal((B_, H, N, D), dtype=np.float32)
    table = rng.standard_normal((961, H), dtype=np.float32)
    index = rng.integers(0, 961, size=(N * N,)).astype(np.int64)
    mask = rng.standard_normal((NW, N, N), dtype=np.float32)
    o = kernel(q=q, k=k, v=v, table=table, index=index, mask=mask)
    print("out", o.shape, o.dtype, float(np.abs(o).mean()))


# revision 21
# speedup vs baseline: 1.1912x; 1.1912x over previous
"""Trainium2 Bass kernel for Swin-style windowed cosine attention.

Problem: nn_Attention_8100308321041
  q,k,v: [512, 8, 256, 16] f32; table: [961, 8]; index: [65536] i64;
  mask: [64, 256, 256] f32; out: [512, 256, 128] f32.

Strategy (8 NeuronCores, pure data-parallel):
  - Shard window-instances b by (b % 64) % 8 == core -> 64 instances/core,
    ordered (wl, img) so each per-window bias+mask table chunk is fetched
    once and reused across 8 images.
  - Host prep: l2-normalize q/k -> bf16 4-head row-group layout (partition
    32*g + d), concatenated with v_aug (ones column -> AV emits numerators
    AND softmax denominators; final divide on HOST, raw av dump shipped
    back bf16).  Per-window tables: CbE = exp(bias+mask) bf16 (pairs
    0,1,3), Cp = round(A*(bias+mask)+B) int16 (pairs 2,3).
  - Device per instance, exp paths balanced across engines (PE kept at its
    4640-cycle floor -- no identity preloads):
      pair0: MULT_ACT  ScalarE exp(S) + GpSimd bf16 multiply by exp(C)
      pair1: MULT_ACT  ScalarE exp(S) + VectorE bf16 2x multiply
      pair2: STT       VectorE Schraudolph int16 bitcast (C-add fused)
      pair3: STT even insts / MULT_ACT (VectorE mult) odd
    out dump copy on ScalarE; qkv + out + C DMA issue on SP queue.
"""

import os
import sys

sys.path.insert(0, "/opt/trn_rl_repo")

import numpy as np
import ml_dtypes

import concourse.bass as bass
import concourse.bacc as bacc
import concourse.mybir as mybir
from concourse import tile
from concourse.bass_utils import run_bass_kernel_spmd

BF16 = ml_dtypes.bfloat16

B_, H, N, D = 512, 8, 256, 16
NW = 64          # windows per image
M_CORES = 8
IMG = B_ // NW   # 8 images
WL = NW // M_CORES  # 8 distinct windows per core
NI = IMG * WL    # 64 instances per core
HD = H * D       # 128
EPS = 1e-12
CBE_WL = 3 * 2 * 2 * N   # exp(C) cols per wl: pairs 0,1,3        (3072)
CP_WL = 2 * 2 * 2 * N    # Schraudolph C' cols per wl: pairs 2,3  (2048)
QKC = 2 * 2 * N          # qk cols per inst (1024)
VC = 2 * H * 17          # v_aug cols per inst (272)
A16 = 128.0 / float(np.log(2.0))     # Schraudolph scale for bf16-via-int16
B16 = 127.0 * 128.0 - 5.09           # Schraudolph bias (round-to-nearest c)

# per-pair exp paths
MULT_ACT, STT = 1, 2

_NC_CACHE = {}


def _paths(inst):
    return [
        MULT_ACT,
        MULT_ACT,
        STT,
        STT if inst % 2 == 0 else MULT_ACT,
    ]


def _cbe_off(wl, pr):
    slot = {0: 0, 1: 1, 3: 2}[pr]
    return wl * CBE_WL + slot * 1024


def _cp_off(wl, pr):
    assert pr in (2, 3)
    return wl * CP_WL + (pr - 2) * 1024


def build_bass(trace_sim=False):
    nc = bacc.Bacc("TRN2", target_bir_lowering=False, debug=False, num_devices=M_CORES)
    qkv = nc.declare_dram_parameter("qkv", [NI, 128, QKC + VC], mybir.dt.bfloat16, isOutput=False)
    CbE = nc.declare_dram_parameter("CbE", [128, WL * CBE_WL], mybir.dt.bfloat16, isOutput=False)
    Cp = nc.declare_dram_parameter("Cp", [128, WL * CP_WL], mybir.dt.int16, isOutput=False)
    out = nc.declare_dram_parameter("out", [NI, 128, VC], mybir.dt.bfloat16, isOutput=True)

    FP32 = mybir.dt.float32
    BF = mybir.dt.bfloat16
    I16 = mybir.dt.int16
    Exp = mybir.ActivationFunctionType.Exp
    Copy = mybir.ActivationFunctionType.Copy

    with tile.TileContext(nc, trace_sim=trace_sim) as tc:
        with (
            tc.tile_pool(name="const", bufs=1) as constp,
            tc.tile_pool(name="qkv", bufs=4) as qkvp,
            tc.tile_pool(name="pp", bufs=10) as ppool,
            tc.tile_pool(name="p0", bufs=6) as p0pool,
            tc.tile_pool(name="op", bufs=3) as opool,
            tc.tile_pool(name="ps", bufs=4, space=bass.MemorySpace.PSUM) as psp,
        ):
            cetile = constp.tile([128, WL * CBE_WL], BF)
            cptile = constp.tile([128, WL * CP_WL], I16)

            def fetch_c(wl):
                nc.sync.dma_start(cetile[:, wl * CBE_WL:(wl + 1) * CBE_WL], CbE[:, wl * CBE_WL:(wl + 1) * CBE_WL])
                nc.sync.dma_start(cptile[:, wl * CP_WL:(wl + 1) * CP_WL], Cp[:, wl * CP_WL:(wl + 1) * CP_WL])

            fetch_c(0)
            fetch_c(1)

            pending = []  # deferred AV + out work from the previous instance

            def emit_av(p_state):
                (p_inst, p_vt, p_pb, avps) = p_state
                for pr in range(4):
                    pbf = p_pb[pr]
                    for hh in range(2):
                        h = 2 * pr + hh
                        hoff = hh * 512
                        for nck in range(2):
                            for mc in range(2):
                                nc.tensor.matmul(
                                    avps[:, nck * (H * 17) + h * 17: nck * (H * 17) + h * 17 + 17],
                                    pbf[:, hoff + mc * 256 + nck * 128: hoff + mc * 256 + nck * 128 + 128],
                                    p_vt[:, mc * (H * 17) + h * 17: mc * (H * 17) + h * 17 + 17],
                                    start=(mc == 0), stop=(mc == 1),
                                )

            def emit_out(p_state):
                # bf16 copy of numerators+denominators on ScalarE; host divides.
                (p_inst, p_vt, p_pb, avps) = p_state
                otile = opool.tile([128, VC], BF, name="otile")
                nc.scalar.activation(otile[:], avps, Copy)
                nc.sync.dma_start(out[p_inst], otile[:])

            def fetch_inst(i):
                t = qkvp.tile([128, QKC + VC], BF, name="qkvtile")
                nc.sync.dma_start(t[:], qkv[i])
                return t

            inst_tiles = {0: fetch_inst(0), 1: fetch_inst(1)}

            for inst in range(NI):
                wl = inst // IMG
                if inst % IMG == 0 and wl + 2 < WL:
                    fetch_c(wl + 2)
                qkvtile = inst_tiles.pop(inst)
                if inst + 2 < NI:
                    inst_tiles[inst + 2] = fetch_inst(inst + 2)
                qk5 = qkvtile[:, 0:QKC].rearrange("p (s q n) -> p s q n", s=2, q=2)
                vtile = qkvtile[:, QKC:QKC + VC]

                paths = _paths(inst)

                pstiles = []
                for pr in range(4):
                    ps = psp.tile([128, 1024], FP32, name="ps", tag="ps")
                    pstiles.append(ps)
                avps_full = psp.tile([128, 1024], FP32, name="avps", tag="ps")

                def qk_burst(half):
                    for mc in range(2):
                        for g in range(4):
                            h = 4 * half + g
                            pr = h // 2
                            hoff = (h % 2) * 512
                            qkh = qk5[32 * g: 32 * g + D, half]
                            nc.tensor.matmul(
                                pstiles[pr][:, hoff + mc * 256: hoff + mc * 256 + 256],
                                qkh[:, 1, mc * 128:(mc + 1) * 128],
                                qkh[:, 0, :],
                                start=(mc == 0),
                                stop=(mc == 1),
                                skip_group_check=True,
                                tile_position=(32 * g, 0),
                            )

                def evac(pr):
                    if paths[pr] == STT:
                        ptile = ppool.tile([128, 1024], I16, name="pt", tag="pt")
                        nc.vector.scalar_tensor_tensor(
                            ptile[:], pstiles[pr][:], A16,
                            cptile[:, _cp_off(wl, pr): _cp_off(wl, pr) + 1024],
                            mybir.AluOpType.mult, mybir.AluOpType.add,
                        )
                        return ptile[:].bitcast(BF)
                    # MULT_ACT: ScalarE exp, then bf16 multiply by exp(C).
                    # pair0's multiply runs on GpSimd (SBUF-only op), the
                    # rest on VectorE.
                    p0tile = p0pool.tile([128, 1024], BF, name="p0t", tag="p0t")
                    nc.scalar.activation(p0tile[:], pstiles[pr][:], Exp)
                    ptile = ppool.tile([128, 1024], BF, name="pt", tag="pt")
                    eng = nc.gpsimd if pr == 0 else nc.vector
                    eng.tensor_mul(
                        ptile[:], p0tile[:],
                        cetile[:, _cbe_off(wl, pr): _cbe_off(wl, pr) + 1024],
                    )
                    return ptile[:]

                # QK half0 (pairs 0,1), evac them, then previous instance's
                # AV + out (before QK half1 so the pool rotation can never
                # deadlock), then QK half1 (pairs 2,3) + evac
                qk_burst(0)
                ptiles = [None] * 4
                ptiles[0] = evac(0)
                ptiles[1] = evac(1)
                if pending:
                    emit_av(pending[0])
                    emit_out(pending[0])
                    pending.clear()
                qk_burst(1)
                ptiles[2] = evac(2)
                ptiles[3] = evac(3)

                pending.append((inst, vtile, ptiles, avps_full[:, 0:VC]))

            if pending:
                emit_av(pending[0])
                emit_out(pending[0])
                pending.clear()
    nc.compile()
    return nc


def _host_prep(q, k, v, table, index, mask):
    """Returns per-core input maps + the inverse b-index map."""
    qn = q / np.maximum(np.sqrt((q * q).sum(-1, keepdims=True)), EPS)
    kn = k / np.maximum(np.sqrt((k * k).sum(-1, keepdims=True)), EPS)
    # 4-head row-group layout: [b, g, d(padded to 32), half, qk, n], h = 4*half+g
    qk8 = np.zeros((B_, 4, 32, 2, 2, N), np.float32)
    qk8[:, :, :D, :, 0] = qn.transpose(0, 1, 3, 2).reshape(B_, 2, 4, D, N).transpose(0, 2, 3, 1, 4)
    qk8[:, :, :D, :, 1] = kn.transpose(0, 1, 3, 2).reshape(B_, 2, 4, D, N).transpose(0, 2, 3, 1, 4)
    qk8 = qk8.reshape(B_, 128, QKC)
    # v_aug [b, n, h, 17] -> [b, mc, 128, h, 17] -> [b, 128, mc*h*17]
    vA = np.empty((B_, N, H, 17), np.float32)
    vA[..., :16] = v.transpose(0, 2, 1, 3)
    vA[..., 16] = 1.0
    vA = vA.reshape(B_, 2, 128, H * 17).transpose(0, 2, 1, 3).reshape(B_, 128, VC)
    qkv = np.concatenate([qk8, vA], axis=2).astype(BF16)  # [B_, 128, 1296]
    # bias'[h, m, n] = table[index[n*256+m], h]
    bias = table[index.astype(np.int64)].reshape(N, N, H).transpose(2, 1, 0)  # [h, m, n]
    maskT = mask.transpose(0, 2, 1)  # [w, m, n]

    in_maps = []
    b_order = []
    for c in range(M_CORES):
        bs = np.array([img * NW + (c + M_CORES * wl) for wl in range(WL) for img in range(IMG)])
        b_order.append(bs)
        C = (bias[None, :, :, :] + maskT[c::M_CORES][:, None, :, :]).astype(np.float32)
        C = C.reshape(WL, H, 2, 128, N)  # [wl, h, mc, 128, n]
        # exp(C) path: pairs {0,1,3} = heads 0,1,2,3,6,7, bf16
        CbE_ = np.exp(C[:, [0, 1, 2, 3, 6, 7]]).transpose(3, 0, 1, 2, 4).reshape(128, WL * CBE_WL).astype(BF16)
        # Schraudolph path: pairs {2,3} = heads 4..7, int16 pre-scaled A*C + B
        Cp_ = np.rint(A16 * C[:, 4:] + B16).transpose(3, 0, 1, 2, 4).reshape(128, WL * CP_WL).astype(np.int16)
        in_maps.append({
            "qkv": np.ascontiguousarray(qkv[bs]),
            "CbE": CbE_,
            "Cp": Cp_,
        })
    return in_maps, b_order


def kernel(q, k, v, table, index, mask):
    q = np.asarray(q, np.float32)
    k = np.asarray(k, np.float32)
    v = np.asarray(v, np.float32)
    table = np.asarray(table, np.float32)
    index = np.asarray(index)
    mask = np.asarray(mask, np.float32)

    in_maps, b_order = _host_prep(q, k, v, table, index, mask)

    if "nc" not in _NC_CACHE:
        _NC_CACHE["nc"] = build_bass()
    nc = _NC_CACHE["nc"]

    res = run_bass_kernel_spmd(nc, in_maps, core_ids=list(range(M_CORES)))
    out = np.empty((B_, N, HD), np.float32)
    for c in range(M_CORES):
        # av dump [NI, 128, (nck h x)] bf16: x = 16 numerators + denominator
        arr = res.results[c]["out"].astype(np.float32).reshape(NI, 128, 2, H, 17)
        o = arr[..., :16] / arr[..., 16:17]           # [NI, p, nck, H, D]
        out[b_order[c]] = o.transpose(0, 2, 1, 3, 4).reshape(NI, N, HD)
    return out


if __name__ == "__main__":
    rng = np.random.default_rng(0)
    q = rng.standard_normal((B_, H, N, D), dtype=np.float32)
    k = rng.standard_normal((B_, H, N, D), dtype=np.float32)
    v = rng.standard_normal((B_, H, N, D), dtype=np.float32)
    table = rng.standard_normal((961, H), dtype=np.float32)
    index = rng.integers(0, 961, size=(N * N,)).astype(np.int64)
    mask = rng.standard_normal((NW, N, N), dtype=np.float32)
    o = kernel(q=q, k=k, v=v, table=table, index=index, mask=mask)
    print("out", o.shape, o.dtype, float(np.abs(o).mean()))
